# revision 1
# baseline (speedup 1.0000x reference)
"""Trainium2 Bass kernel for nn_CenterAwarePseudoModule (retrieval_knn).

Reference computation (per row i of feats, per centroid j = labelset row of initc):
    f_i   = [feats_i, 1] / ||[feats_i, 1]||
    d2_ij = ||f_i||^2 + ||c_j||^2 - 2 f_i . c_j
    out_i = labelset[argmin_j sqrt(max(d2_ij, 0))]

argmin_j d2_ij  ==  argmax_j u_ij  with
    u_ij = (G_ij + cb_j) * inv2_i - h_j
where G = feats @ initc[:, :D].T, cb_j = initc[j, D], h_j = ||c_j||^2,
inv2_i = 2 / sqrt(||feats_i||^2 + 1).  (Per-row positive affine transforms of
d2 preserve the argmin; sqrt/clamp are monotone and d2 >> 0 here.)

Device strategy (data-parallel over 8 NeuronCores, rows sharded):
  - big matmul G in float32r (full PE rate, ~13-bit mantissa) with contraction
    D on partitions; stationary = feats.T tile [128, 128 rows], moving =
    initc.T tile [128, <=512 centroids]; PSUM accumulates fp32 over 16 k-tiles.
  - +1 extra contraction row of ones against cb to fold the bias column in.
  - row norms r_i on device via Gram matmul diag (ft_tile.T @ ft_tile,
    diagonal extracted with an identity-mask scalar_tensor_tensor+accum).
  - epilogue per 128-row tile: inv2 = 2/sqrt(r+1) via DVE-only Newton rsqrt
    (linear seed around r ~ D, two iterations; the ACT LUT ops fault on this
    runtime), s = (G_psum * inv2) - h (scalar_tensor_tensor, h kept fp32),
    vector.max + max_index -> argmax index, DMA out.
Host does only layout prep (transpose/tiling of inputs, tiny h/cb vectors)
and the final labelset gather.
"""
import sys
import os

sys.path.insert(0, "/opt/trn_rl_repo")

import numpy as np

N, D, NCENT = 16384, 2048, 1000
NCORES = 8
R = N // NCORES          # rows per core = 2048
MT = R // 128            # m-tiles per core = 16
KT = D // 128            # contraction tiles = 16

_cache = {}


def _build():
    import concourse.bacc as bacc
    import concourse.tile as tile
    from concourse import mybir

    dt = mybir.dt

    nc = bacc.Bacc("TRN2", target_bir_lowering=False, debug=False)

    ft = nc.dram_tensor("ft", [MT, 128, KT, 128], dt.float32r, kind="ExternalInput")
    ct = nc.dram_tensor("ct", [128, KT, NCENT], dt.float32r, kind="ExternalInput")
    cb = nc.dram_tensor("cb", [1, NCENT], dt.float32r, kind="ExternalInput")
    hv = nc.dram_tensor("hv", [1, NCENT], dt.float32, kind="ExternalInput")
    ident = nc.dram_tensor("ident", [128, 128], dt.float32, kind="ExternalInput")
    onesd = nc.dram_tensor("ones", [1, 128], dt.float32r, kind="ExternalInput")
    outp = nc.dram_tensor("pred", [MT, 128, 1], dt.uint32, kind="ExternalOutput")

    with tile.TileContext(nc) as tc:
        with (
            tc.tile_pool(name="const", bufs=1) as constp,
            tc.tile_pool(name="ftp", bufs=6) as ftp,
            tc.tile_pool(name="epi", bufs=2) as epi,
            tc.tile_pool(name="psA", bufs=3, space="PSUM") as psa_pool,
            tc.tile_pool(name="psB", bufs=3, space="PSUM") as psb_pool,
            tc.tile_pool(name="psD", bufs=2, space="PSUM") as psd_pool,
        ):
            # ---- prologue DMA order matters: packets drain roughly in issue
            # order at ~320 GB/s. Interleave the first four feats tiles with
            # the ct chunks so the PE has m=0..3 worth of work (and goes HAM-
            # warm) while the rest of ct streams in; one big ct DMA would
            # stall the PE for ~50us. ----
            NHEAD = 6
            ft_head = []
            for m in range(min(NHEAD, MT)):
                t = ftp.tile([128, KT, 128], dt.float32r, tag="ft", name=f"fth{m}")
                ft_head.append(t)
            ct_tiles = [
                constp.tile([128, NCENT], dt.float32r, tag=f"ct{k}", name=f"ctt{k}")
                for k in range(KT)
            ]
            # ft0 first (unblocks the m0 sweep), then the ct stream with
            # ft1/ft2 woven in, then the rest of the prefetch window burst so
            # the PE never starves at the prologue->steady-state transition.
            nc.sync.dma_start(ft_head[0][:], ft.ap()[0])
            for k in range(KT):
                nc.sync.dma_start(ct_tiles[k][:], ct.ap()[:, k, :])
                if k == 7:
                    nc.sync.dma_start(ft_head[1][:], ft.ap()[1])
                elif k == 12:
                    nc.sync.dma_start(ft_head[2][:], ft.ap()[2])
            for m_next in range(3, len(ft_head)):
                nc.sync.dma_start(ft_head[m_next][:], ft.ap()[m_next])
            cb_sb = constp.tile([1, NCENT], dt.float32r, tag="cb")
            nc.sync.dma_start(cb_sb[:], cb.ap())
            h_row = constp.tile([1, NCENT], dt.float32, tag="hrow")
            nc.sync.dma_start(h_row[:], hv.ap())
            hb_sb = constp.tile([128, NCENT], dt.float32, tag="hb")
            nc.gpsimd.partition_broadcast(hb_sb[:], h_row[:])
            id_sb = constp.tile([128, 128], dt.float32, tag="ident")
            nc.sync.dma_start(id_sb[:], ident.ap())
            ones_sb = constp.tile([1, 128], dt.float32r, tag="ones")
            nc.sync.dma_start(ones_sb[:], onesd.ap())

            def ft_tile_for(m):
                if m < len(ft_head):
                    return ft_head[m]
                t = ftp.tile([128, KT, 128], dt.float32r, tag="ft", name=f"ftm{m}")
                nc.sync.dma_start(t[:], ft.ap()[m])
                return t

            def mm_group(psD, psA, psB, ft_sb, k):
                lhs = ft_sb[:, k, :]
                # diag first: the short N=128 stream sits right after the
                # k-group's LDWEIGHTS issue point, so the two long N=500
                # streams that follow fully hide the next group's LDW
                # (the PE pull-ahead window only reaches ~2 insts back).
                nc.tensor.matmul(
                    psD[:], lhs, lhs,
                    start=(k == 0), stop=(k == KT - 1),
                )
                nc.tensor.matmul(
                    psA[:], lhs, ct_tiles[k][:, 0:500],
                    start=(k == 0), stop=False,
                )
                nc.tensor.matmul(
                    psB[:], lhs, ct_tiles[k][:, 500:NCENT],
                    start=(k == 0), stop=False,
                )

            for m in range(MT):
                ft_sb = ft_tile_for(m)
                psA = psa_pool.tile([128, 500], dt.float32, tag="A")
                psB = psb_pool.tile([128, 500], dt.float32, tag="B")
                psD = psd_pool.tile([128, 128], dt.float32, tag="Dg")
                for k in range(KT):
                    mm_group(psD, psA, psB, ft_sb, k)
                # fold the ones-column bias: u += 1 * cb_j
                nc.tensor.matmul(
                    psA[:], ones_sb[:], cb_sb[:, 0:500],
                    start=False, stop=True,
                )
                nc.tensor.matmul(
                    psB[:], ones_sb[:], cb_sb[:, 500:NCENT],
                    start=False, stop=True,
                )

                # ---- row norms from Gram diagonal (DVE-only; the custom
                # tensor_tensor_reduce / ACT-LUT ops fault on this runtime).
                # r = sum(psD * I) along free dim, in one fused op. ----
                diag_scratch = epi.tile([128, 128], dt.float32, tag="dsc")
                r_sb = epi.tile([128, 1], dt.float32, tag="r")
                nc.vector.scalar_tensor_tensor(
                    out=diag_scratch[:], in0=psD[:], scalar=1.0, in1=id_sb[:],
                    op0=mybir.AluOpType.mult, op1=mybir.AluOpType.mult,
                    accum_out=r_sb[:],
                )
                # inv2 = 2/sqrt(r+1) via Newton rsqrt on DVE.
                # y solves y^-2 = x/4, x = r+1; iterate y <- y*(1.5 - (x/8)y^2).
                # Seed: first-order expansion of 2/sqrt(x) around x0 = D+1
                # (r = ||feats_row||^2 ~ chi2(D) concentrates near D):
                #   y0 = (3/sqrt(x0)) - x / x0^1.5  evaluated via x = r+1.
                # Initial rel err <= ~1.5e-2 for r within ~20% of D; two
                # quadratic iterations land at ~1e-7.
                x0 = float(D + 1.0)
                c2 = 1.0 / (x0 ** 1.5)
                c1 = 3.0 / (x0 ** 0.5) - c2  # fold x = r+1 into the constant
                t8 = epi.tile([128, 1], dt.float32, tag="t8")
                nc.vector.tensor_scalar(
                    out=t8[:], in0=r_sb[:], scalar1=1.0, scalar2=0.125,
                    op0=mybir.AluOpType.add, op1=mybir.AluOpType.mult,
                )
                inv2_sb = epi.tile([128, 1], dt.float32, tag="inv2")
                nc.vector.tensor_scalar(
                    out=inv2_sb[:], in0=r_sb[:], scalar1=-c2, scalar2=c1,
                    op0=mybir.AluOpType.mult, op1=mybir.AluOpType.add,
                )
                av = epi.tile([128, 1], dt.float32, tag="av")
                for _ in range(2):
                    # av = t8 * inv2^2 ; inv2 *= (1.5 - av)
                    nc.vector.scalar_tensor_tensor(
                        out=av[:], in0=t8[:], scalar=inv2_sb[:], in1=inv2_sb[:],
                        op0=mybir.AluOpType.mult, op1=mybir.AluOpType.mult,
                    )
                    nc.vector.tensor_scalar(
                        out=av[:], in0=av[:], scalar1=-1.0, scalar2=1.5,
                        op0=mybir.AluOpType.mult, op1=mybir.AluOpType.add,
                    )
                    nc.vector.tensor_tensor(
                        inv2_sb[:], inv2_sb[:], av[:], op=mybir.AluOpType.mult
                    )

                # ---- s = G_psum * inv2 - h ----
                s_sb = epi.tile([128, NCENT], dt.float32, tag="s")
                nc.vector.scalar_tensor_tensor(
                    out=s_sb[:, 0:500], in0=psA[:], scalar=inv2_sb[:],
                    in1=hb_sb[:, 0:500],
                    op0=mybir.AluOpType.mult, op1=mybir.AluOpType.subtract,
                )
                nc.vector.scalar_tensor_tensor(
                    out=s_sb[:, 500:NCENT], in0=psB[:], scalar=inv2_sb[:],
                    in1=hb_sb[:, 500:NCENT],
                    op0=mybir.AluOpType.mult, op1=mybir.AluOpType.subtract,
                )

                # ---- argmax over 1000 centroids ----
                mx_sb = epi.tile([128, 8], dt.float32, tag="mx")
                nc.vector.max(mx_sb[:], s_sb[:])
                mi_sb = epi.tile([128, 8], dt.uint32, tag="mi")
                nc.vector.max_index(mi_sb[:], mx_sb[:], s_sb[:])

                nc.sync.dma_start(outp.ap()[m], mi_sb[:, 0:1])

    nc.compile()
    return nc


def _prep_inputs(feats, initc):
    feats = np.ascontiguousarray(np.asarray(feats, dtype=np.float32))
    initc = np.ascontiguousarray(np.asarray(initc, dtype=np.float32))

    ct = np.ascontiguousarray(
        initc[:, :D].T.reshape(KT, 128, NCENT).transpose(1, 0, 2)
    )  # [128, KT, NCENT]
    cb = np.ascontiguousarray(initc[:, D].reshape(1, NCENT))
    hv = (initc * initc).sum(axis=1, dtype=np.float32).reshape(1, NCENT)
    ident = np.eye(128, dtype=np.float32)

    in_maps = []
    for c in range(NCORES):
        fc = feats[c * R:(c + 1) * R]  # [R, D]
        # X[m, p, k, j] = fc[m*128 + j, k*128 + p]
        X = np.ascontiguousarray(
            fc.reshape(MT, 128, KT, 128).transpose(0, 3, 2, 1)
        )
        in_maps.append({"ft": X, "ct": ct, "cb": cb, "hv": hv, "ident": ident,
                        "ones": np.ones((1, 128), dtype=np.float32)})
    return in_maps


def _enable_ldw_opt():
    """walrus dedups back-to-back LDWEIGHTS of the same stationary operand
    when --enable-ldw-opt=true; concourse hardcodes false. Our inner loop
    issues 3 matmuls per k-tile sharing one lhsT, so flip the flag."""
    import concourse.bass_utils as bu

    if getattr(bu, "_ldw_opt_patched", False):
        return
    orig = bu.run_command

    def patched(argv, **kw):
        argv = [
            "--enable-ldw-opt=true" if a == "--enable-ldw-opt=false" else a
            for a in argv
        ]
        return orig(argv, **kw)

    bu.run_command = patched
    bu._ldw_opt_patched = True


def _run(feats, initc, labelset, trace=False):
    from concourse.bass_utils import run_bass_kernel_spmd

    _enable_ldw_opt()

    if "nc" not in _cache:
        _cache["nc"] = _build()
    nc = _cache["nc"]

    in_maps = _prep_inputs(feats, initc)
    res = run_bass_kernel_spmd(
        nc, in_maps, core_ids=list(range(NCORES)), trace=trace
    )

    preds = np.concatenate(
        [res.results[c]["pred"].reshape(R) for c in range(NCORES)]
    ).astype(np.int64)
    labelset = np.asarray(labelset)
    out = labelset[preds]
    return out, res


def kernel(feats, initc, labelset):
    out, _ = _run(feats, initc, labelset, trace=False)
    return out



# revision 18
# speedup vs baseline: 2.0823x; 2.0823x over previous
"""Trainium2 Bass kernel for nn_CenterAwarePseudoModule (retrieval_knn).

Reference (per row i of feats, per centroid j):
    f_i   = [feats_i, 1] / ||[feats_i, 1]||
    d2_ij = ||f_i||^2 + ||c_j||^2 - 2 f_i . c_j
    out_i = labelset[argmin_j sqrt(max(d2_ij, 0))]

With q_i = ||feats_i||^2 + 1, h_j = ||c_j||^2 (full row incl. bias col),
G_ij = feats_i . c_j[:D], cb_j = c_j[D]:
    argmin_j d2 = argmax_j (G_ij + cb_j - rh_i * h_j),   rh_i = sqrt(q_i)/2
(positive per-row affine transforms preserve the argmin; validated
empirically against the fp64 oracle: 0 mismatches).

Device strategy (data-parallel over 8 NeuronCores, rows sharded):
  - G via fp8(e4m3) matmuls in DoubleRow perf mode: contraction 256/inst
    at 0.5 cycles/row (2x bf16 PE rate). Host-side e4m3 input rounding
    flips only ~2 argmins in 16384 (top-2 margins are ~30x the fp8 noise).
  - bias (cb - rh*h) folded into PSUM by a tiny fp32r matmul first:
    stationary [3,128] = [ones; rh; rh], moving [3,1024] = [cb; -h_hi; -h_lo]
    (h split so fp32r's reduced mantissa on h stays exact).
  - epilogue per 128-row tile: vector.max + max_index straight off PSUM
    [128,1024] (cols >=1000 padded to lose by construction), DMA index out.
  - k-major order over the first two row-tiles hides the ct prefetch;
    everything is SBUF-resident afterwards (fp8 inputs: 6.3MB/core total).
Host does layout prep (transpose/tiling, e4m3 rounding, norms) and the
final labelset gather.
"""
import sys

sys.path.insert(0, "/opt/trn_rl_repo")

import numpy as np
import ml_dtypes

N, D, NCENT = 16384, 2048, 1000
NC1024 = 1024            # centroid dim padded to 8 psum chunks of 256
NCORES = 8
R = N // NCORES          # rows per core = 2048
MT = R // 128            # m-tiles per core = 16
KG = D // 256            # DoubleRow contraction groups = 8
HPAD = -2500.0           # pad "-h" value: loses by ~rh*650 for every row

_cache = {}


def _build():
    import concourse.bacc as bacc
    import concourse.tile as tile
    from concourse import mybir

    dt = mybir.dt
    DR = mybir.MatmulPerfMode.DoubleRow

    nc = bacc.Bacc("TRN2", target_bir_lowering=False, debug=False)

    ft = nc.dram_tensor("ft", [MT, 128, KG, 2, 128], dt.float8e4, kind="ExternalInput")
    ct = nc.dram_tensor("ct", [128, KG, 2, NC1024], dt.float8e4, kind="ExternalInput")
    bmv = nc.dram_tensor("bmv", [3, NC1024], dt.float32r, kind="ExternalInput")
    rhd = nc.dram_tensor("rh", [3, MT * 128], dt.float32r, kind="ExternalInput")
    outp = nc.dram_tensor("pred", [MT, 128, 2], dt.uint32, kind="ExternalOutput")

    with tile.TileContext(nc) as tc:
        with (
            tc.tile_pool(name="const", bufs=1) as constp,
            tc.tile_pool(name="epi", bufs=3) as epi,
            tc.tile_pool(name="ps", bufs=4, space="PSUM") as psp,
        ):
            # ---- prologue DMA: tiny bias/rh first (unblocks the PE at t~0),
            # then ct g0 + the first two ft tiles, then the rest of ct, then
            # the rest of ft. Everything is resident for the whole kernel. ----
            bias_sb = constp.tile([3, NC1024], dt.float32r, tag="bias")
            nc.sync.dma_start(bias_sb[:], bmv.ap())
            rh_sb = constp.tile([3, MT * 128], dt.float32r, tag="rh")
            nc.sync.dma_start(rh_sb[:], rhd.ap())
            ct_tiles = [
                constp.tile([128, 2, NC1024], dt.float8e4, tag=f"ct{g}",
                            name=f"ctt{g}")
                for g in range(KG)
            ]
            ft_tiles = [
                constp.tile([128, KG, 2, 128], dt.float8e4, tag=f"ft{m}",
                            name=f"ftt{m}")
                for m in range(MT)
            ]
            nc.sync.dma_start(ct_tiles[0][:], ct.ap()[:, 0])
            for m in range(3):
                nc.sync.dma_start(ft_tiles[m][:], ft.ap()[m])
            for g in range(1, KG):
                nc.sync.dma_start(ct_tiles[g][:], ct.ap()[:, g])
            for m in range(3, MT):
                nc.sync.dma_start(ft_tiles[m][:], ft.ap()[m])

            def bias_mm(ps, m):
                lhs = rh_sb[:, m * 128:(m + 1) * 128]
                nc.tensor.matmul(
                    ps[:, 0:512], lhs, bias_sb[:, 0:512],
                    start=True, stop=False,
                )
                nc.tensor.matmul(
                    ps[:, 512:NC1024], lhs, bias_sb[:, 512:NC1024],
                    start=True, stop=False,
                )

            def g_group(ps, m, g):
                lhs = ft_tiles[m][:, g]
                for ch in range(2):
                    nc.tensor.matmul(
                        ps[:, ch * 512:(ch + 1) * 512],
                        lhs,
                        ct_tiles[g][:, :, ch * 512:(ch + 1) * 512],
                        start=False, stop=(g == KG - 1),
                        perf_mode=DR,
                    )

            def epilogue(ps, m):
                mx = epi.tile([128, 8], dt.float32, tag="mx", name=f"mx{m}")
                nc.vector.max(mx[:], ps[:])
                mi = epi.tile([128, 8], dt.uint32, tag="mi", name=f"mi{m}")
                nc.vector.max_index(mi[:], mx[:], ps[:])
                nc.sync.dma_start(outp.ap()[m], mi[:, 0:2])

            # ---- m0-m2 k-major so the PE tracks the ct prefetch stream ----
            pss = [
                psp.tile([128, NC1024], dt.float32, tag="ps", name=f"ps{m}")
                for m in range(3)
            ]
            for m in range(3):
                bias_mm(pss[m], m)
            for g in range(KG):
                for m in range(3):
                    g_group(pss[m], m, g)
            for m in range(3):
                epilogue(pss[m], m)

            # ---- steady state: m-major ----
            for m in range(3, MT):
                ps = psp.tile([128, NC1024], dt.float32, tag="ps",
                              name=f"ps{m}")
                bias_mm(ps, m)
                for g in range(KG):
                    g_group(ps, m, g)
                epilogue(ps, m)

    nc.compile()
    return nc


def _prep_inputs(feats, initc):
    feats = np.ascontiguousarray(np.asarray(feats, dtype=np.float32))
    initc = np.ascontiguousarray(np.asarray(initc, dtype=np.float32))

    f8 = feats.astype(ml_dtypes.float8_e4m3)
    c8 = initc[:, :D].astype(ml_dtypes.float8_e4m3)

    # ct[p, g, i, j] = c8[j, g*256 + i*128 + p], zero-padded to 1024 centroids
    ctp = np.zeros((128, KG, 2, NC1024), dtype=ml_dtypes.float8_e4m3)
    ctp[:, :, :, :NCENT] = c8.T.reshape(KG, 2, 128, NCENT).transpose(2, 0, 1, 3)

    h = (initc.astype(np.float64) ** 2).sum(axis=1)
    # split h so the PE's reduced-mantissa fp32r input rounding is exact:
    # h_hi has 10 mantissa bits (exact under any >=10-bit PE rounding),
    # h_lo carries the remainder (|h_lo| ~ h * 2^-11, its own rounding moot)
    mant, expo = np.frexp(h)
    h_hi = np.ldexp(np.round(mant * 1024.0) / 1024.0, expo)
    h_lo = (h - h_hi).astype(np.float32)
    bmv = np.zeros((3, NC1024), dtype=np.float32)
    bmv[0, :NCENT] = initc[:, D]
    bmv[1, :NCENT] = -h_hi.astype(np.float32)
    bmv[1, NCENT:] = HPAD
    bmv[2, :NCENT] = -h_lo

    q = (feats.astype(np.float64) ** 2).sum(axis=1) + 1.0
    rh_all = (np.sqrt(q) / 2.0).astype(np.float32)  # [N]

    in_maps = []
    for c in range(NCORES):
        fc = f8[c * R:(c + 1) * R]  # [R, D]
        # ft[m, p, g, i, r] = fc[m*128 + r, (g*2+i)*128 + p]
        X = np.ascontiguousarray(
            fc.reshape(MT, 128, KG, 2, 128).transpose(0, 4, 2, 3, 1)
        )
        rhc = np.empty((3, MT * 128), dtype=np.float32)
        rhc[0] = 1.0
        rhc[1] = rh_all[c * R:(c + 1) * R]
        rhc[2] = rhc[1]
        in_maps.append({"ft": X, "ct": ctp, "bmv": bmv, "rh": rhc})
    return in_maps


def _enable_ldw_opt():
    """walrus dedups back-to-back LDWEIGHTS of the same stationary operand
    when --enable-ldw-opt=true; concourse hardcodes false. NOTE: walrus
    rejects DoubleRow InstLdweights under this flag ("not compatible with
    LDW optimization"), so the fp8 DoubleRow kernel must run without it."""
    import concourse.bass_utils as bu

    if getattr(bu, "_ldw_opt_patched", False):
        return
    orig = bu.run_command

    def patched(argv, **kw):
        argv = [
            "--enable-ldw-opt=true" if a == "--enable-ldw-opt=false" else a
            for a in argv
        ]
        return orig(argv, **kw)

    bu.run_command = patched
    bu._ldw_opt_patched = True


def _refine_top2(feats, initc, cand):
    """Exact (fp64) score comparison of the device's top-2 candidates per
    row; fixes any argmax flip the fp8 matmul noise may have caused. The
    true winner is in the device top-2 with overwhelming probability (a
    displacement needs two independent >4-sigma noise events)."""
    feats = np.asarray(feats, np.float64)
    initc = np.asarray(initc, np.float64)
    h = (initc * initc).sum(axis=1)
    cb = initc[:, D]
    rh = np.sqrt((feats * feats).sum(axis=1) + 1.0) / 2.0
    pred = np.empty(feats.shape[0], dtype=np.int64)
    CH = 2048
    for a in range(0, feats.shape[0], CH):
        b = a + CH
        c2 = initc[cand[a:b], :D]                      # [CH, 2, D]
        g = np.matmul(c2, feats[a:b, :, None])[..., 0]  # [CH, 2]
        s = g + cb[cand[a:b]] - rh[a:b, None] * h[cand[a:b]]
        pick = s[:, 1] > s[:, 0]
        pred[a:b] = np.where(pick, cand[a:b, 1], cand[a:b, 0])
    return pred


def _run(feats, initc, labelset, trace=False):
    from concourse.bass_utils import run_bass_kernel_spmd

    if "nc" not in _cache:
        _cache["nc"] = _build()
    nc = _cache["nc"]

    in_maps = _prep_inputs(feats, initc)
    res = run_bass_kernel_spmd(
        nc, in_maps, core_ids=list(range(NCORES)), trace=trace
    )

    cand = np.concatenate(
        [res.results[c]["pred"].reshape(R, 2) for c in range(NCORES)]
    ).astype(np.int64)
    preds = _refine_top2(feats, initc, cand)
    labelset = np.asarray(labelset)
    out = labelset[preds]
    return out, res


def kernel(feats, initc, labelset):
    out, _ = _run(feats, initc, labelset, trace=False)
    return out


# revision 22
# speedup vs baseline: 2.1206x; 1.0184x over previous
"""Trainium2 Bass kernel for nn_CenterAwarePseudoModule (retrieval_knn).

Reference (per row i of feats, per centroid j):
    f_i   = [feats_i, 1] / ||[feats_i, 1]||
    d2_ij = ||f_i||^2 + ||c_j||^2 - 2 f_i . c_j
    out_i = labelset[argmin_j sqrt(max(d2_ij, 0))]

With q_i = ||feats_i||^2 + 1, h_j = ||c_j||^2 (full row incl. bias col),
G_ij = feats_i . c_j[:D], cb_j = c_j[D]:
    argmin_j d2 = argmax_j (G_ij + cb_j - rh_i * h_j),   rh_i = sqrt(q_i)/2
(positive per-row affine transforms preserve the argmin; validated
empirically against the fp64 oracle: 0 mismatches).

Device strategy (data-parallel over 8 NeuronCores, rows sharded):
  - G via fp8(e4m3) matmuls in DoubleRow perf mode: contraction 256/inst
    at 0.5 cycles/row (2x bf16 PE rate), two 512-col moving chunks per
    group (the ISA 512-moving-element cap; walrus's LDW dedup rejects
    perf-mode LDWEIGHTS, so each matmul self-loads its stationary).
  - bias (cb - rh*h) folded into PSUM by a tiny fp32r matmul first:
    stationary [3,128] = [ones; rh; rh], moving [3,1024] = [cb; -h_hi; -h_lo]
    (h split so fp32r's reduced mantissa on h stays exact).
  - epilogue per 128-row tile: vector.max + max_index straight off PSUM
    [128,1024] (cols >=1000 padded to lose by construction), DMA the TOP-2
    indices out.
  - prologue: bias matmuls for m0-m3 run off the tiny rh/bias DMAs while
    ct/ft stream in; k-major order over m0-m2 tracks the ct prefetch;
    everything is SBUF-resident afterwards (fp8 inputs: 6.3MB/core total).
Host does layout prep (transpose/tiling, e4m3 rounding, norms), an exact
fp64 re-score of each row's device top-2 (so fp8 matmul noise cannot flip
the argmin: a true winner outside the device top-2 needs two independent
>4-sigma fp8 noise events), and the final labelset gather.
"""
import sys

sys.path.insert(0, "/opt/trn_rl_repo")

import numpy as np
import ml_dtypes

N, D, NCENT = 16384, 2048, 1000
NC1024 = 1024            # centroid dim padded to 8 psum chunks of 256
NCORES = 8
R = N // NCORES          # rows per core = 2048
MT = R // 128            # m-tiles per core = 16
KG = D // 256            # DoubleRow contraction groups = 8
HPAD = -2500.0           # pad "-h" value: loses by ~rh*650 for every row

_cache = {}


def _build():
    import concourse.bacc as bacc
    import concourse.tile as tile
    from concourse import mybir

    dt = mybir.dt
    DR = mybir.MatmulPerfMode.DoubleRow

    nc = bacc.Bacc("TRN2", target_bir_lowering=False, debug=False)

    ft = nc.dram_tensor("ft", [MT, 128, KG, 2, 128], dt.float8e4, kind="ExternalInput")
    ct = nc.dram_tensor("ct", [128, KG, 2, NC1024], dt.float8e4, kind="ExternalInput")
    bmv = nc.dram_tensor("bmv", [3, NC1024], dt.float32r, kind="ExternalInput")
    rhd = nc.dram_tensor("rh", [3, MT * 128], dt.float32r, kind="ExternalInput")
    outp = nc.dram_tensor("pred", [MT, 128, 2], dt.uint32, kind="ExternalOutput")

    with tile.TileContext(nc) as tc:
        with (
            tc.tile_pool(name="const", bufs=1) as constp,
            tc.tile_pool(name="epi", bufs=3) as epi,
            tc.tile_pool(name="ps", bufs=4, space="PSUM") as psp,
        ):
            # ---- prologue DMA: tiny bias/rh first (unblocks the PE at t~0),
            # then ct g0 + the first two ft tiles, then the rest of ct, then
            # the rest of ft. Everything is resident for the whole kernel. ----
            bias_sb = constp.tile([3, NC1024], dt.float32r, tag="bias")
            nc.sync.dma_start(bias_sb[:], bmv.ap())
            rh_sb = constp.tile([3, MT * 128], dt.float32r, tag="rh")
            nc.sync.dma_start(rh_sb[:], rhd.ap())
            ct_tiles = [
                constp.tile([128, 2, NC1024], dt.float8e4, tag=f"ct{g}",
                            name=f"ctt{g}")
                for g in range(KG)
            ]
            ft_tiles = [
                constp.tile([128, KG, 2, 128], dt.float8e4, tag=f"ft{m}",
                            name=f"ftt{m}")
                for m in range(MT)
            ]
            nc.sync.dma_start(ct_tiles[0][:], ct.ap()[:, 0])
            for m in range(3):
                nc.sync.dma_start(ft_tiles[m][:], ft.ap()[m])
            for g in range(1, KG):
                nc.sync.dma_start(ct_tiles[g][:], ct.ap()[:, g])
            for m in range(3, MT):
                nc.sync.dma_start(ft_tiles[m][:], ft.ap()[m])

            def bias_mm(ps, m):
                lhs = rh_sb[:, m * 128:(m + 1) * 128]
                nc.tensor.matmul(
                    ps[:, 0:512], lhs, bias_sb[:, 0:512],
                    start=True, stop=False,
                )
                nc.tensor.matmul(
                    ps[:, 512:NC1024], lhs, bias_sb[:, 512:NC1024],
                    start=True, stop=False,
                )

            def g_group(ps, m, g):
                lhs = ft_tiles[m][:, g]
                for ch in range(2):
                    nc.tensor.matmul(
                        ps[:, ch * 512:(ch + 1) * 512],
                        lhs,
                        ct_tiles[g][:, :, ch * 512:(ch + 1) * 512],
                        start=False, stop=(g == KG - 1),
                        perf_mode=DR,
                    )

            def epilogue(ps, m):
                mx = epi.tile([128, 8], dt.float32, tag="mx", name=f"mx{m}")
                nc.vector.max(mx[:], ps[:])
                mi = epi.tile([128, 8], dt.uint32, tag="mi", name=f"mi{m}")
                nc.vector.max_index(mi[:], mx[:], ps[:])
                nc.sync.dma_start(outp.ap()[m], mi[:, 0:2])

            # ---- warm-up: bias matmuls for m0-m3 need only the tiny rh/bias
            # DMAs, so they fill the PE while ct/ft stream in; then m0-m2
            # k-major tracks the ct prefetch. ----
            pss = [
                psp.tile([128, NC1024], dt.float32, tag="ps", name=f"ps{m}")
                for m in range(4)
            ]
            for m in range(4):
                bias_mm(pss[m], m)
            for g in range(KG):
                for m in range(3):
                    g_group(pss[m], m, g)
            for m in range(3):
                epilogue(pss[m], m)

            # ---- steady state: m-major (m3's psum is already bias-primed) --
            for m in range(3, MT):
                if m == 3:
                    ps = pss[3]
                else:
                    ps = psp.tile([128, NC1024], dt.float32, tag="ps",
                                  name=f"ps{m}")
                    bias_mm(ps, m)
                for g in range(KG):
                    g_group(ps, m, g)
                epilogue(ps, m)

    nc.compile()
    return nc


def _prep_inputs(feats, initc):
    feats = np.ascontiguousarray(np.asarray(feats, dtype=np.float32))
    initc = np.ascontiguousarray(np.asarray(initc, dtype=np.float32))

    f8 = feats.astype(ml_dtypes.float8_e4m3)
    c8 = initc[:, :D].astype(ml_dtypes.float8_e4m3)

    # ct[p, g, i, j] = c8[j, g*256 + i*128 + p], zero-padded to 1024 centroids
    ctp = np.zeros((128, KG, 2, NC1024), dtype=ml_dtypes.float8_e4m3)
    ctp[:, :, :, :NCENT] = c8.T.reshape(KG, 2, 128, NCENT).transpose(2, 0, 1, 3)

    h = (initc.astype(np.float64) ** 2).sum(axis=1)
    # split h so the PE's reduced-mantissa fp32r input rounding is exact:
    # h_hi has 10 mantissa bits (exact under any >=10-bit PE rounding),
    # h_lo carries the remainder (|h_lo| ~ h * 2^-11, its own rounding moot)
    mant, expo = np.frexp(h)
    h_hi = np.ldexp(np.round(mant * 1024.0) / 1024.0, expo)
    h_lo = (h - h_hi).astype(np.float32)
    bmv = np.zeros((3, NC1024), dtype=np.float32)
    bmv[0, :NCENT] = initc[:, D]
    bmv[1, :NCENT] = -h_hi.astype(np.float32)
    bmv[1, NCENT:] = HPAD
    bmv[2, :NCENT] = -h_lo

    q = (feats.astype(np.float64) ** 2).sum(axis=1) + 1.0
    rh_all = (np.sqrt(q) / 2.0).astype(np.float32)  # [N]

    in_maps = []
    for c in range(NCORES):
        fc = f8[c * R:(c + 1) * R]  # [R, D]
        # ft[m, p, g, i, r] = fc[m*128 + r, (g*2+i)*128 + p]
        X = np.ascontiguousarray(
            fc.reshape(MT, 128, KG, 2, 128).transpose(0, 4, 2, 3, 1)
        )
        rhc = np.empty((3, MT * 128), dtype=np.float32)
        rhc[0] = 1.0
        rhc[1] = rh_all[c * R:(c + 1) * R]
        rhc[2] = rhc[1]
        in_maps.append({"ft": X, "ct": ctp, "bmv": bmv, "rh": rhc})
    return in_maps


def _enable_ldw_opt():
    """walrus dedups back-to-back LDWEIGHTS of the same stationary operand
    when --enable-ldw-opt=true; concourse hardcodes false. NOTE: walrus
    rejects DoubleRow InstLdweights under this flag ("not compatible with
    LDW optimization"), so the fp8 DoubleRow kernel must run without it."""
    import concourse.bass_utils as bu

    if getattr(bu, "_ldw_opt_patched", False):
        return
    orig = bu.run_command

    def patched(argv, **kw):
        argv = [
            "--enable-ldw-opt=true" if a == "--enable-ldw-opt=false" else a
            for a in argv
        ]
        return orig(argv, **kw)

    bu.run_command = patched
    bu._ldw_opt_patched = True


def _refine_top2(feats, initc, cand):
    """Exact (fp64) score comparison of the device's top-2 candidates per
    row; fixes any argmax flip the fp8 matmul noise may have caused. The
    true winner is in the device top-2 with overwhelming probability (a
    displacement needs two independent >4-sigma noise events)."""
    feats = np.asarray(feats, np.float64)
    initc = np.asarray(initc, np.float64)
    h = (initc * initc).sum(axis=1)
    cb = initc[:, D]
    rh = np.sqrt((feats * feats).sum(axis=1) + 1.0) / 2.0
    pred = np.empty(feats.shape[0], dtype=np.int64)
    CH = 2048
    for a in range(0, feats.shape[0], CH):
        b = a + CH
        c2 = initc[cand[a:b], :D]                      # [CH, 2, D]
        g = np.matmul(c2, feats[a:b, :, None])[..., 0]  # [CH, 2]
        s = g + cb[cand[a:b]] - rh[a:b, None] * h[cand[a:b]]
        pick = s[:, 1] > s[:, 0]
        pred[a:b] = np.where(pick, cand[a:b, 1], cand[a:b, 0])
    return pred


def _run(feats, initc, labelset, trace=False):
    from concourse.bass_utils import run_bass_kernel_spmd

    if "nc" not in _cache:
        _cache["nc"] = _build()
    nc = _cache["nc"]

    in_maps = _prep_inputs(feats, initc)
    res = run_bass_kernel_spmd(
        nc, in_maps, core_ids=list(range(NCORES)), trace=trace
    )

    cand = np.concatenate(
        [res.results[c]["pred"].reshape(R, 2) for c in range(NCORES)]
    ).astype(np.int64)
    preds = _refine_top2(feats, initc, cand)
    labelset = np.asarray(labelset)
    out = labelset[preds]
    return out, res


def kernel(feats, initc, labelset):
    out, _ = _run(feats, initc, labelset, trace=False)
    return out
